# revision 7
# baseline (speedup 1.0000x reference)
"""Trainium2 Bass kernel for nn_Encoder_55362128445616.

Transformer encoder layer: B=8, S=1024, D=512, single-head attention over
H*D=4096. Sharding: data-parallel over batch, one batch element per core,
no collectives.

Key algebraic folding (host-side, exact):
  scores = Q K^T / s = x (Wq^T Wk / s) x^T  (+ per-k bias from bq; per-q
  terms cancel in softmax), so the 4096-dim QKV contractions collapse to
  512-dim ones via M = Wq^T Wk / s and NT = (Wo Wv)^T.  LN1's affine is
  folded into W1/b1; all matmuls run as fp32r (1 cycle/row on PE).
"""

import math

import numpy as np

import concourse.bacc as bacc
import concourse.mybir as mybir
import concourse.tile as tile
from concourse import masks
from concourse.bass_utils import run_bass_kernel_spmd

B, S, D = 8, 1024, 512
NQ = S // 128   # 8 q/k tiles of 128
ND = D // 128   # 4 d tiles of 128
F32 = mybir.dt.float32
F32R = mybir.dt.float32r
AF = mybir.ActivationFunctionType
AX = mybir.AxisListType

_BUILT = None  # (nc,) cache


def _build():
    global _BUILT
    if _BUILT is not None:
        return _BUILT

    nc = bacc.Bacc("TRN2", target_bir_lowering=False, debug=False, num_devices=B)

    def din(name, shape, dt=F32R):
        return nc.dram_tensor(name, shape, dt, kind="ExternalInput").ap()

    xT_d = din("xT", [D, S])              # x[b].T
    xn_d = din("xn", [S, D])              # x[b]
    xres_d = din("xres", [S, D], F32)     # x[b] + (Wo@bv + bo)
    M_d = din("Mw", [D, D])               # Wq^T Wk / sqrt(D)
    NT_d = din("NT", [D, D])              # (Wo @ Wv)^T
    W1gT_d = din("W1gT", [D, D])          # (W1 * g0)^T
    W2T_d = din("W2T", [D, D])            # W2^T
    abias_d = din("abias", [NQ, 128, 1], F32)  # (x @ (Wk^T bq))/sqrt(D)
    c1_d = din("c1", [ND, 128, 1], F32)   # b1 + W1 @ be0
    C2_d = din("C2", [1, D])              # b2 + be0
    onesc_d = din("onesc", [128, 2])
    onesr_d = din("onesr", [1, 128])
    g0b_d = din("g0b", [128, D], F32)
    g1b_d = din("g1b", [128, D], F32)
    be1b_d = din("be1b", [128, D], F32)
    out_d = nc.dram_tensor("out", [S, D], F32, kind="ExternalOutput").ap()

    with tile.TileContext(nc) as tc:
        with (
            tc.tile_pool(name="res", bufs=1) as res,
            tc.tile_pool(name="work", bufs=2) as work,
            tc.tile_pool(name="small", bufs=4) as small,
            tc.tile_pool(name="psA", bufs=3, space="PSUM") as psA,
            tc.tile_pool(name="psS", bufs=2, space="PSUM") as psS,
            tc.tile_pool(name="psD", bufs=1, space="PSUM") as psD,
        ):
            # ---- resident loads ----
            xT = res.tile([128, ND, S], F32R, tag="zx")   # xT[d,:][p,t,q]: d=t*128+p
            nc.sync.dma_start(xT[:], xT_d.rearrange("(t p) q -> p t q", p=128))
            Mw = res.tile([128, ND, D], F32R)
            nc.sync.dma_start(Mw[:], M_d.rearrange("(t p) n -> p t n", p=128))
            xn = res.tile([128, NQ, D], F32R)
            nc.sync.dma_start(xn[:], xn_d.rearrange("(t p) n -> p t n", p=128))
            NTw = res.tile([128, ND, D], F32R)
            nc.sync.dma_start(NTw[:], NT_d.rearrange("(t p) n -> p t n", p=128))
            xres = res.tile([128, NQ, D], F32)
            nc.sync.dma_start(xres[:], xres_d.rearrange("(t p) n -> p t n", p=128))
            W1gT = res.tile([128, ND, D], F32R)
            nc.sync.dma_start(W1gT[:], W1gT_d.rearrange("(t p) n -> p t n", p=128))
            W2T = res.tile([128, ND, D], F32R)
            nc.sync.dma_start(W2T[:], W2T_d.rearrange("(t p) n -> p t n", p=128))
            abias = res.tile([128, NQ], F32)
            nc.sync.dma_start(abias[:], abias_d.rearrange("t p o -> p (t o)"))
            c1 = res.tile([128, ND], F32)
            nc.sync.dma_start(c1[:], c1_d.rearrange("t p o -> p (t o)"))
            C2 = res.tile([1, D], F32R)
            nc.sync.dma_start(C2[:], C2_d[:])
            onesc = res.tile([128, 2], F32R)
            nc.sync.dma_start(onesc[:], onesc_d[:])
            onesr = res.tile([1, 128], F32R)
            nc.sync.dma_start(onesr[:], onesr_d[:])
            g0b = res.tile([128, D], F32)
            nc.sync.dma_start(g0b[:], g0b_d[:])
            g1b = res.tile([128, D], F32)
            nc.sync.dma_start(g1b[:], g1b_d[:])
            be1b = res.tile([128, D], F32)
            nc.sync.dma_start(be1b[:], be1b_d[:])

            ident = res.tile([128, 128], F32)
            masks.make_identity(nc, ident[:])
            epsT = res.tile([128, 1], F32)
            nc.gpsimd.memset(epsT[:], 1e-5)

            # big SBUF intermediates; "mid" and "zx" tags reuse slots
            IN2 = res.tile([128, ND, S], F32R, tag="mid")   # M^T x^T
            PT = res.tile([128, NQ, S], F32R)               # exp(scores^T)
            zT = res.tile([128, ND, S], F32R)
            ff1T = res.tile([128, ND, S], F32R)

            # ---- phase 1: IN2[a, q] = sum_b M[b, a] xT[b, q] ----
            for at in range(ND):
                for qc in range(2):
                    ps = psA.tile([128, 512], F32, tag="a")
                    for bt in range(ND):
                        nc.tensor.matmul(
                            ps[:],
                            Mw[:, bt, at * 128:(at + 1) * 128],
                            xT[:, bt, qc * 512:(qc + 1) * 512],
                            start=(bt == 0), stop=(bt == ND - 1),
                        )
                    nc.scalar.copy(IN2[:, at, qc * 512:(qc + 1) * 512], ps[:])

            # ---- phase 2: scoresT[k, q] = sum_a x[k, a] IN2[a, q]; PT=exp ----
            for kt in range(NQ):
                ps = psS.tile([128, 1024], F32, tag="s")
                for qc in range(2):
                    for at in range(ND):
                        nc.tensor.matmul(
                            ps[:, qc * 512:(qc + 1) * 512],
                            xT[:, at, kt * 128:(kt + 1) * 128],
                            IN2[:, at, qc * 512:(qc + 1) * 512],
                            start=(at == 0), stop=(at == ND - 1),
                        )
                nc.scalar.activation(
                    PT[:, kt, :], ps[:], AF.Exp, bias=abias[:, kt:kt + 1]
                )

            # denomT[q] = sum_k PT[k, q], computed directly in [q, 1] form:
            # per q-tile column, accumulate PT[:, kt, q-slice].T @ ones over kt.
            dps = psD.tile([128, 2 * NQ], F32, tag="d")
            for qt in range(NQ):
                for kt in range(NQ):
                    nc.tensor.matmul(
                        dps[:, 2 * qt:2 * qt + 2],
                        PT[:, kt, qt * 128:(qt + 1) * 128],
                        onesc[:],
                        start=(kt == 0), stop=(kt == NQ - 1),
                    )
            recip = res.tile([128, 2 * NQ], F32)
            nc.vector.reciprocal(recip[:], dps[:])

            # ---- phase 3: ST[d, q] = sum_k x[k, d] PT[k, q] ----
            ST = res.tile([128, ND, S], F32R, tag="mid")
            for dt in range(ND):
                for qc in range(2):
                    ps = psA.tile([128, 512], F32, tag="a")
                    for kt in range(NQ):
                        nc.tensor.matmul(
                            ps[:],
                            xn[:, kt, dt * 128:(dt + 1) * 128],
                            PT[:, kt, qc * 512:(qc + 1) * 512],
                            start=(kt == 0), stop=(kt == NQ - 1),
                        )
                    nc.scalar.copy(ST[:, dt, qc * 512:(qc + 1) * 512], ps[:])

            # ---- phase 4: mha + residual + LN1 -> z ----
            z = res.tile([128, NQ, D], F32, tag="zx")
            for qt in range(NQ):
                ps = psA.tile([128, 512], F32, tag="a")
                for dt in range(ND):
                    nc.tensor.matmul(
                        ps[:],
                        ST[:, dt, qt * 128:(qt + 1) * 128],
                        NTw[:, dt, :],
                        start=(dt == 0), stop=(dt == ND - 1),
                    )
                x1 = work.tile([128, D], F32, tag="mha")
                nc.scalar.mul(x1[:], ps[:], recip[:, 2 * qt:2 * qt + 1])
                nc.vector.tensor_add(x1[:], x1[:], xres[:, qt, :])
                _layernorm_to(nc, small, work, x1, z[:, qt, :], epsT)

            # ---- phase 5: zT = z^T (PE transposes) ----
            for qt in range(NQ):
                for dt in range(ND):
                    pst = psA.tile([128, 128], F32, tag="a", name="pst")
                    nc.tensor.transpose(
                        pst[:], z[:, qt, dt * 128:(dt + 1) * 128], ident[:]
                    )
                    nc.scalar.copy(zT[:, dt, qt * 128:(qt + 1) * 128], pst[:])

            # ---- phase 6: ff1T[e, q] = relu(sum_d W1gT[d, e] zT[d, q] + c1[e]) ----
            for et in range(ND):
                for qc in range(2):
                    ps = psA.tile([128, 512], F32, tag="a")
                    for dt in range(ND):
                        nc.tensor.matmul(
                            ps[:],
                            W1gT[:, dt, et * 128:(et + 1) * 128],
                            zT[:, dt, qc * 512:(qc + 1) * 512],
                            start=(dt == 0), stop=(dt == ND - 1),
                        )
                    nc.scalar.activation(
                        ff1T[:, et, qc * 512:(qc + 1) * 512], ps[:],
                        AF.Relu, bias=c1[:, et:et + 1],
                    )

            # ---- phase 7: ff2 + bias + residual + LN2 -> out ----
            for qt in range(NQ):
                ps = psA.tile([128, 512], F32, tag="a")
                for et in range(ND):
                    nc.tensor.matmul(
                        ps[:],
                        ff1T[:, et, qt * 128:(qt + 1) * 128],
                        W2T[:, et, :],
                        start=(et == 0), stop=False,
                    )
                nc.tensor.matmul(ps[:], onesr[:], C2[:], start=False, stop=True)
                hres = work.tile([128, D], F32, tag="hres")
                nc.vector.tensor_mul(hres[:], z[:, qt, :], g0b[:])
                r = work.tile([128, D], F32, tag="r")
                nc.vector.tensor_add(r[:], ps[:], hres[:])
                z2 = work.tile([128, D], F32, tag="mha")
                _layernorm_to(nc, small, work, r, z2[:], epsT)
                z2g = work.tile([128, D], F32, tag="sq")
                nc.vector.tensor_mul(z2g[:], z2[:], g1b[:])
                ot = work.tile([128, D], F32, tag="r")
                nc.vector.tensor_add(ot[:], z2g[:], be1b[:])
                nc.sync.dma_start(
                    out_d.rearrange("(t p) n -> p t n", p=128)[:, qt, :], ot[:]
                )

    nc.compile()
    _BUILT = (nc,)
    return _BUILT


def _layernorm_to(nc, small, work, x1, out_ap, epsT):
    """out = (x1 - mean(x1)) * rsqrt(var(x1) + 1e-5), over the free axis (512)."""
    s1 = small.tile([128, 1], F32, tag="s1")
    nc.vector.reduce_sum(s1[:], x1[:], axis=AX.X)
    nm = small.tile([128, 1], F32, tag="nm")
    nc.scalar.mul(nm[:], s1[:], -1.0 / D)
    sq = work.tile([128, D], F32, tag="sq")
    sqs = small.tile([128, 1], F32, tag="sqs")
    nc.scalar.activation(sq[:], x1[:], AF.Square, bias=nm[:], accum_out=sqs[:])
    stdv = small.tile([128, 1], F32, tag="stdv")
    nc.scalar.activation(stdv[:], sqs[:], AF.Sqrt, scale=1.0 / D, bias=epsT[:])
    rstd = small.tile([128, 1], F32, tag="rstd")
    nc.vector.reciprocal(rstd[:], stdv[:])
    nmr = small.tile([128, 1], F32, tag="nmr")
    nc.vector.tensor_mul(nmr[:], nm[:], rstd[:])
    nc.scalar.activation(out_ap, x1[:], AF.Identity, scale=rstd[:], bias=nmr[:])


def _prepare_in_maps(inputs):
    f64 = np.float64
    g = {k: np.asarray(v) for k, v in inputs.items()}
    x = g["x"].astype(f64)
    Wq, Wk, Wv = g["Wq"].astype(f64), g["Wk"].astype(f64), g["Wv"].astype(f64)
    Wo, W1, W2 = g["Wo"].astype(f64), g["W1"].astype(f64), g["W2"].astype(f64)
    bq, bk, bv, bo = g["bq"].astype(f64), g["bk"].astype(f64), g["bv"].astype(f64), g["bo"].astype(f64)
    b1, b2 = g["b1"].astype(f64), g["b2"].astype(f64)
    g0, be0, g1, be1 = g["g0"].astype(f64), g["be0"].astype(f64), g["g1"].astype(f64), g["be1"].astype(f64)

    s = math.sqrt(D)
    f32 = lambda a: np.ascontiguousarray(a, dtype=np.float32)

    Mw = f32(Wq.T @ Wk / s)
    NT = f32((Wo @ Wv).T)
    W1gT = f32((W1 * g0[None, :]).T)
    W2T = f32(W2.T)
    c1 = f32(b1 + W1 @ be0).reshape(ND, 128, 1)
    C2 = f32(b2 + be0).reshape(1, D)
    wbo = Wo @ bv + bo
    vk = Wk.T @ bq / s
    g0b = f32(np.broadcast_to(g0, (128, D)))
    g1b = f32(np.broadcast_to(g1, (128, D)))
    be1b = f32(np.broadcast_to(be1, (128, D)))
    onesc = np.ones((128, 2), np.float32)
    onesr = np.ones((1, 128), np.float32)

    shared = dict(Mw=Mw, NT=NT, W1gT=W1gT, W2T=W2T, c1=c1, C2=C2,
                  onesc=onesc, onesr=onesr, g0b=g0b, g1b=g1b, be1b=be1b)
    in_maps = []
    for b in range(B):
        xb = x[b]
        m = dict(shared)
        m["xT"] = f32(xb.T)
        m["xn"] = f32(xb)
        m["xres"] = f32(xb + wbo[None, :])
        m["abias"] = f32(xb @ vk).reshape(NQ, 128, 1)
        in_maps.append(m)
    return in_maps


def _run(inputs, trace=False):
    (nc,) = _build()
    in_maps = _prepare_in_maps(inputs)
    res = run_bass_kernel_spmd(nc, in_maps, core_ids=list(range(B)), trace=trace)
    out = np.stack([res.results[c]["out"] for c in range(B)]).astype(np.float32)
    return out, res


def kernel(**inputs):
    out, _ = _run(inputs, trace=False)
    return out


# revision 8
# speedup vs baseline: 1.0803x; 1.0803x over previous
"""Trainium2 Bass kernel for nn_Encoder_55362128445616.

Transformer encoder layer: B=8, S=1024, D=512, single-head attention over
H*D=4096. Sharding: data-parallel over batch, one batch element per core,
no collectives.

Key algebraic folding (host-side, exact):
  scores = Q K^T / s = x (Wq^T Wk / s) x^T  (+ per-k bias from bq; per-q
  terms cancel in softmax), so the 4096-dim QKV contractions collapse to
  512-dim ones via M = Wq^T Wk / s and NT = (Wo Wv)^T.  LN1's affine is
  folded into W1/b1; all matmuls run as fp32r (1 cycle/row on PE).
"""

import math

import numpy as np

import concourse.bacc as bacc
import concourse.mybir as mybir
import concourse.tile as tile
from concourse import masks
from concourse.bass_utils import run_bass_kernel_spmd

B, S, D = 8, 1024, 512
NQ = S // 128   # 8 q/k tiles of 128
ND = D // 128   # 4 d tiles of 128
F32 = mybir.dt.float32
F32R = mybir.dt.float32r
AF = mybir.ActivationFunctionType
AX = mybir.AxisListType

_BUILT = None  # (nc,) cache


def _build():
    global _BUILT
    if _BUILT is not None:
        return _BUILT

    nc = bacc.Bacc("TRN2", target_bir_lowering=False, debug=False, num_devices=B)

    def din(name, shape, dt=F32R):
        return nc.dram_tensor(name, shape, dt, kind="ExternalInput").ap()

    xT_d = din("xT", [D, S])              # x[b].T
    xn_d = din("xn", [S, D])              # x[b]
    xres_d = din("xres", [S, D], F32)     # x[b] + (Wo@bv + bo)
    M_d = din("Mw", [D, D])               # Wq^T Wk / sqrt(D)
    NT_d = din("NT", [D, D])              # (Wo @ Wv)^T
    W1gT_d = din("W1gT", [D, D])          # (W1 * g0)^T
    W2T_d = din("W2T", [D, D])            # W2^T
    abias_d = din("abias", [NQ, 128, 1], F32)  # (x @ (Wk^T bq))/sqrt(D)
    c1_d = din("c1", [ND, 128, 1], F32)   # b1 + W1 @ be0
    C2_d = din("C2", [1, D])              # b2 + be0
    onesc_d = din("onesc", [128, 2], F32)
    onesr_d = din("onesr", [1, 128])
    g0b_d = din("g0b", [128, D], F32)
    g1b_d = din("g1b", [128, D], F32)
    be1b_d = din("be1b", [128, D], F32)
    out_d = nc.dram_tensor("out", [S, D], F32, kind="ExternalOutput").ap()

    with tile.TileContext(nc) as tc:
        with (
            tc.tile_pool(name="res", bufs=1) as res,
            tc.tile_pool(name="work", bufs=2) as work,
            tc.tile_pool(name="small", bufs=4) as small,
            tc.tile_pool(name="psA", bufs=3, space="PSUM") as psA,
            tc.tile_pool(name="psS", bufs=2, space="PSUM") as psS,
            tc.tile_pool(name="psD", bufs=1, space="PSUM") as psD,
        ):
            # ---- resident loads ----
            # sync (HWDGE) carries the phase-1/2 critical path in use-order;
            # gpsimd (SWDGE) carries later-phase tensors in parallel.
            Mw = res.tile([128, ND, D], F32R)
            nc.sync.dma_start(Mw[:], M_d.rearrange("(t p) n -> p t n", p=128))
            xT = res.tile([128, ND, S], F32R, tag="zx")   # xT[d,:][p,t,q]: d=t*128+p
            nc.sync.dma_start(xT[:], xT_d.rearrange("(t p) q -> p t q", p=128))
            abias = res.tile([128, NQ], F32)
            nc.sync.dma_start(abias[:], abias_d.rearrange("t p o -> p (t o)"))
            onesc = res.tile([128, 2], F32)
            nc.sync.dma_start(onesc[:], onesc_d[:])
            xn = res.tile([128, NQ, D], F32R)
            nc.sync.dma_start(xn[:], xn_d.rearrange("(t p) n -> p t n", p=128))

            NTw = res.tile([128, ND, D], F32R)
            nc.gpsimd.dma_start(NTw[:], NT_d.rearrange("(t p) n -> p t n", p=128))
            xres = res.tile([128, NQ, D], F32)
            nc.gpsimd.dma_start(xres[:], xres_d.rearrange("(t p) n -> p t n", p=128))
            W1gT = res.tile([128, ND, D], F32R)
            nc.gpsimd.dma_start(W1gT[:], W1gT_d.rearrange("(t p) n -> p t n", p=128))
            W2T = res.tile([128, ND, D], F32R)
            nc.gpsimd.dma_start(W2T[:], W2T_d.rearrange("(t p) n -> p t n", p=128))
            c1 = res.tile([128, ND], F32)
            nc.gpsimd.dma_start(c1[:], c1_d.rearrange("t p o -> p (t o)"))
            C2 = res.tile([1, D], F32R)
            nc.gpsimd.dma_start(C2[:], C2_d[:])
            onesr = res.tile([1, 128], F32R)
            nc.gpsimd.dma_start(onesr[:], onesr_d[:])
            g0b = res.tile([128, D], F32)
            nc.gpsimd.dma_start(g0b[:], g0b_d[:])
            g1b = res.tile([128, D], F32)
            nc.gpsimd.dma_start(g1b[:], g1b_d[:])
            be1b = res.tile([128, D], F32)
            nc.gpsimd.dma_start(be1b[:], be1b_d[:])

            ident = res.tile([128, 128], F32)
            masks.make_identity(nc, ident[:])
            epsT = res.tile([128, 1], F32)
            nc.gpsimd.memset(epsT[:], 1e-5)

            # HAM warm-up: keep PE busy while the first DMAs land.
            for w in range(16):
                psw = psA.tile([128, 128], F32, tag="a", name=f"psw{w}")
                nc.tensor.matmul(psw[:], ident[:], ident[:], start=True, stop=True)

            # big SBUF intermediates; "mid" and "zx" tags reuse slots
            IN2 = res.tile([128, ND, S], F32R, tag="mid")   # M^T x^T
            PT = res.tile([128, NQ, S], F32R)               # exp(scores^T)
            zT = res.tile([128, ND, S], F32R)
            ff1T = res.tile([128, ND, S], F32R)

            # ---- phase 1: IN2[a, q] = sum_b M[b, a] xT[b, q] ----
            for at in range(ND):
                for qc in range(2):
                    ps = psA.tile([128, 512], F32, tag="a")
                    for bt in range(ND):
                        nc.tensor.matmul(
                            ps[:],
                            Mw[:, bt, at * 128:(at + 1) * 128],
                            xT[:, bt, qc * 512:(qc + 1) * 512],
                            start=(bt == 0), stop=(bt == ND - 1),
                        )
                    nc.vector.tensor_copy(IN2[:, at, qc * 512:(qc + 1) * 512], ps[:])

            # ---- phase 2: scoresT[k, q] = sum_a x[k, a] IN2[a, q]; PT=exp ----
            for kt in range(NQ):
                ps = psS.tile([128, 1024], F32, tag="s")
                for qc in range(2):
                    for at in range(ND):
                        nc.tensor.matmul(
                            ps[:, qc * 512:(qc + 1) * 512],
                            xT[:, at, kt * 128:(kt + 1) * 128],
                            IN2[:, at, qc * 512:(qc + 1) * 512],
                            start=(at == 0), stop=(at == ND - 1),
                        )
                nc.scalar.activation(
                    PT[:, kt, :], ps[:], AF.Exp, bias=abias[:, kt:kt + 1]
                )

            # denomT[q] = sum_k PT[k, q]: DVE tree-sums the 8 k-tiles, then one
            # cheap f32 ones-matmul per q-tile flips [k-part, q] to [q-part, 1].
            dps = psD.tile([128, 2 * NQ], F32, tag="d")
            for qc in range(2):
                qs = slice(qc * 512, (qc + 1) * 512)
                dacc = work.tile([128, 512], F32, tag="sq", name=f"dacc{qc}")
                nc.vector.tensor_add(dacc[:], PT[:, 0, qs].bitcast(F32), PT[:, 1, qs].bitcast(F32))
                for kt in range(2, NQ):
                    nc.vector.tensor_add(dacc[:], dacc[:], PT[:, kt, qs].bitcast(F32))
                for ql in range(4):
                    qt = qc * 4 + ql
                    nc.tensor.matmul(
                        dps[:, 2 * qt:2 * qt + 2],
                        dacc[:, ql * 128:(ql + 1) * 128],
                        onesc[:],
                        start=True, stop=True,
                    )
            recip = res.tile([128, 2 * NQ], F32)
            nc.vector.reciprocal(recip[:], dps[:])

            # ---- phase 3: ST[d, q] = sum_k x[k, d] PT[k, q] ----
            ST = res.tile([128, ND, S], F32R, tag="mid")
            for dt in range(ND):
                for qc in range(2):
                    ps = psA.tile([128, 512], F32, tag="a")
                    for kt in range(NQ):
                        nc.tensor.matmul(
                            ps[:],
                            xn[:, kt, dt * 128:(dt + 1) * 128],
                            PT[:, kt, qc * 512:(qc + 1) * 512],
                            start=(kt == 0), stop=(kt == NQ - 1),
                        )
                    nc.scalar.copy(ST[:, dt, qc * 512:(qc + 1) * 512], ps[:])

            # ---- phase 4: mha + residual + LN1 -> z ----
            z = res.tile([128, NQ, D], F32, tag="zx")
            for qt in range(NQ):
                ps = psA.tile([128, 512], F32, tag="a")
                for dt in range(ND):
                    nc.tensor.matmul(
                        ps[:],
                        ST[:, dt, qt * 128:(qt + 1) * 128],
                        NTw[:, dt, :],
                        start=(dt == 0), stop=(dt == ND - 1),
                    )
                x1 = work.tile([128, D], F32, tag="mha")
                nc.scalar.mul(x1[:], ps[:], recip[:, 2 * qt:2 * qt + 1])
                nc.vector.tensor_add(x1[:], x1[:], xres[:, qt, :])
                _layernorm_to(nc, small, work, x1, z[:, qt, :], epsT)

            # ---- phase 5: zT = z^T (PE transposes) ----
            for qt in range(NQ):
                for dt in range(ND):
                    pst = psA.tile([128, 128], F32, tag="a", name="pst")
                    nc.tensor.transpose(
                        pst[:], z[:, qt, dt * 128:(dt + 1) * 128], ident[:]
                    )
                    nc.vector.tensor_copy(zT[:, dt, qt * 128:(qt + 1) * 128], pst[:])

            # ---- phase 6: ff1T[e, q] = relu(sum_d W1gT[d, e] zT[d, q] + c1[e]) ----
            for et in range(ND):
                for qc in range(2):
                    ps = psA.tile([128, 512], F32, tag="a")
                    for dt in range(ND):
                        nc.tensor.matmul(
                            ps[:],
                            W1gT[:, dt, et * 128:(et + 1) * 128],
                            zT[:, dt, qc * 512:(qc + 1) * 512],
                            start=(dt == 0), stop=(dt == ND - 1),
                        )
                    nc.scalar.activation(
                        ff1T[:, et, qc * 512:(qc + 1) * 512], ps[:],
                        AF.Relu, bias=c1[:, et:et + 1],
                    )

            # ---- phase 7: ff2 + bias + residual + LN2 -> out ----
            for qt in range(NQ):
                ps = psA.tile([128, 512], F32, tag="a")
                for et in range(ND):
                    nc.tensor.matmul(
                        ps[:],
                        ff1T[:, et, qt * 128:(qt + 1) * 128],
                        W2T[:, et, :],
                        start=(et == 0), stop=False,
                    )
                nc.tensor.matmul(ps[:], onesr[:], C2[:], start=False, stop=True)
                hres = work.tile([128, D], F32, tag="hres")
                nc.vector.tensor_mul(hres[:], z[:, qt, :], g0b[:])
                r = work.tile([128, D], F32, tag="r")
                nc.vector.tensor_add(r[:], ps[:], hres[:])
                z2 = work.tile([128, D], F32, tag="mha")
                _layernorm_to(nc, small, work, r, z2[:], epsT)
                z2g = work.tile([128, D], F32, tag="sq")
                nc.vector.tensor_mul(z2g[:], z2[:], g1b[:])
                ot = work.tile([128, D], F32, tag="r")
                nc.vector.tensor_add(ot[:], z2g[:], be1b[:])
                nc.sync.dma_start(
                    out_d.rearrange("(t p) n -> p t n", p=128)[:, qt, :], ot[:]
                )

    nc.compile()
    _BUILT = (nc,)
    return _BUILT


def _layernorm_to(nc, small, work, x1, out_ap, epsT):
    """out = (x1 - mean(x1)) * rsqrt(var(x1) + 1e-5), over the free axis (512)."""
    s1 = small.tile([128, 1], F32, tag="s1")
    nc.vector.reduce_sum(s1[:], x1[:], axis=AX.X)
    nm = small.tile([128, 1], F32, tag="nm")
    nc.scalar.mul(nm[:], s1[:], -1.0 / D)
    sq = work.tile([128, D], F32, tag="sq")
    sqs = small.tile([128, 1], F32, tag="sqs")
    nc.scalar.activation(sq[:], x1[:], AF.Square, bias=nm[:], accum_out=sqs[:])
    stdv = small.tile([128, 1], F32, tag="stdv")
    nc.scalar.activation(stdv[:], sqs[:], AF.Sqrt, scale=1.0 / D, bias=epsT[:])
    rstd = small.tile([128, 1], F32, tag="rstd")
    nc.vector.reciprocal(rstd[:], stdv[:])
    nmr = small.tile([128, 1], F32, tag="nmr")
    nc.vector.tensor_mul(nmr[:], nm[:], rstd[:])
    nc.scalar.activation(out_ap, x1[:], AF.Identity, scale=rstd[:], bias=nmr[:])


def _prepare_in_maps(inputs):
    f64 = np.float64
    g = {k: np.asarray(v) for k, v in inputs.items()}
    x = g["x"].astype(f64)
    Wq, Wk, Wv = g["Wq"].astype(f64), g["Wk"].astype(f64), g["Wv"].astype(f64)
    Wo, W1, W2 = g["Wo"].astype(f64), g["W1"].astype(f64), g["W2"].astype(f64)
    bq, bk, bv, bo = g["bq"].astype(f64), g["bk"].astype(f64), g["bv"].astype(f64), g["bo"].astype(f64)
    b1, b2 = g["b1"].astype(f64), g["b2"].astype(f64)
    g0, be0, g1, be1 = g["g0"].astype(f64), g["be0"].astype(f64), g["g1"].astype(f64), g["be1"].astype(f64)

    s = math.sqrt(D)
    f32 = lambda a: np.ascontiguousarray(a, dtype=np.float32)

    Mw = f32(Wq.T @ Wk / s)
    NT = f32((Wo @ Wv).T)
    W1gT = f32((W1 * g0[None, :]).T)
    W2T = f32(W2.T)
    c1 = f32(b1 + W1 @ be0).reshape(ND, 128, 1)
    C2 = f32(b2 + be0).reshape(1, D)
    wbo = Wo @ bv + bo
    vk = Wk.T @ bq / s
    g0b = f32(np.broadcast_to(g0, (128, D)))
    g1b = f32(np.broadcast_to(g1, (128, D)))
    be1b = f32(np.broadcast_to(be1, (128, D)))
    onesc = np.ones((128, 2), np.float32)
    onesr = np.ones((1, 128), np.float32)

    shared = dict(Mw=Mw, NT=NT, W1gT=W1gT, W2T=W2T, c1=c1, C2=C2,
                  onesc=onesc, onesr=onesr, g0b=g0b, g1b=g1b, be1b=be1b)
    in_maps = []
    for b in range(B):
        xb = x[b]
        m = dict(shared)
        m["xT"] = f32(xb.T)
        m["xn"] = f32(xb)
        m["xres"] = f32(xb + wbo[None, :])
        m["abias"] = f32(xb @ vk).reshape(NQ, 128, 1)
        in_maps.append(m)
    return in_maps


def _run(inputs, trace=False):
    (nc,) = _build()
    in_maps = _prepare_in_maps(inputs)
    res = run_bass_kernel_spmd(nc, in_maps, core_ids=list(range(B)), trace=trace)
    out = np.stack([res.results[c]["out"] for c in range(B)]).astype(np.float32)
    return out, res


def kernel(**inputs):
    out, _ = _run(inputs, trace=False)
    return out


# revision 9
# speedup vs baseline: 1.4373x; 1.3304x over previous
"""Trainium2 Bass kernel for nn_Encoder_55362128445616.

Transformer encoder layer: B=8, S=1024, D=512, single-head attention over
H*D=4096. Sharding: data-parallel over batch, one batch element per core,
no collectives.

Key algebraic folding (host-side, exact):
  scores = Q K^T / s = x (Wq^T Wk / s) x^T  (+ per-k bias from bq; per-q
  terms cancel in softmax), so the 4096-dim QKV contractions collapse to
  512-dim ones via M = Wq^T Wk / s and NT = (Wo Wv)^T.  LN1's affine is
  folded into W1/b1; all matmuls run as fp32r (1 cycle/row on PE).

Two compiled variants: `fast` (biases zero, LN affines identity — matches
the reference's setup_inputs) and a general fallback.
"""

import math

import numpy as np

import concourse.bacc as bacc
import concourse.mybir as mybir
import concourse.tile as tile
from concourse import masks
from concourse.bass_utils import run_bass_kernel_spmd

B, S, D = 8, 1024, 512
NQ = S // 128   # 8 q/k tiles of 128
ND = D // 128   # 4 d tiles of 128
F32 = mybir.dt.float32
F32R = mybir.dt.float32r
AF = mybir.ActivationFunctionType
AX = mybir.AxisListType

_BUILT = {}


def _build(fast):
    if fast in _BUILT:
        return _BUILT[fast]

    nc = bacc.Bacc("TRN2", target_bir_lowering=False, debug=False, num_devices=B)

    def din(name, shape, dt=F32R):
        return nc.dram_tensor(name, shape, dt, kind="ExternalInput").ap()

    xT_d = din("xT", [D, S])              # x[b].T
    xn_d = din("xn", [S, D])              # x[b]
    M_d = din("Mw", [D, D])               # Wq^T Wk / sqrt(D)
    NT_d = din("NT", [D, D])              # (Wo @ Wv)^T
    W1gT_d = din("W1gT", [D, D])          # (W1 * g0)^T
    W2T_d = din("W2T", [D, D])            # W2^T
    # packed per-partition columns: [0:8]=abias, [8:10]=ones, [10:14]=c1, [14:15]=eps
    sm_d = din("smalls", [128, 16], F32)
    if not fast:
        xres_d = din("xres", [S, D], F32)     # x[b] + (Wo@bv + bo)
        C2_d = din("C2", [1, D])              # b2 + be0
        onesr_d = din("onesr", [1, 128])
        g0b_d = din("g0b", [128, D], F32)
        g1b_d = din("g1b", [128, D], F32)
        be1b_d = din("be1b", [128, D], F32)
    out_d = nc.dram_tensor("out", [S, D], F32, kind="ExternalOutput").ap()

    with tile.TileContext(nc) as tc:
        with (
            tc.tile_pool(name="res", bufs=1) as res,
            tc.tile_pool(name="work", bufs=2) as work,
            tc.tile_pool(name="small", bufs=8) as small,
            tc.tile_pool(name="psA", bufs=3, space="PSUM") as psA,
            tc.tile_pool(name="psS", bufs=2, space="PSUM") as psS,
            tc.tile_pool(name="psD", bufs=1, space="PSUM") as psD,
        ):
            # ---- resident loads: sync (HWDGE) carries the phase-1/2 critical
            # path in use-order; gpsimd (SWDGE) carries later-phase tensors. ----
            Mw = res.tile([128, ND, D], F32R)
            nc.sync.dma_start(Mw[:], M_d.rearrange("(t p) n -> p t n", p=128))
            xT = res.tile([128, ND, S], F32R, tag="zx")
            nc.sync.dma_start(xT[:], xT_d.rearrange("(t p) q -> p t q", p=128))
            sm = res.tile([128, 16], F32)
            nc.sync.dma_start(sm[:], sm_d[:])
            xn = res.tile([128, NQ, D], F32R)
            nc.sync.dma_start(xn[:], xn_d.rearrange("(t p) n -> p t n", p=128))

            NTw = res.tile([128, ND, D], F32R)
            nc.gpsimd.dma_start(NTw[:], NT_d.rearrange("(t p) n -> p t n", p=128))
            W1gT = res.tile([128, ND, D], F32R)
            nc.gpsimd.dma_start(W1gT[:], W1gT_d.rearrange("(t p) n -> p t n", p=128))
            W2T = res.tile([128, ND, D], F32R)
            nc.gpsimd.dma_start(W2T[:], W2T_d.rearrange("(t p) n -> p t n", p=128))
            if not fast:
                xres = res.tile([128, NQ, D], F32)
                nc.gpsimd.dma_start(xres[:], xres_d.rearrange("(t p) n -> p t n", p=128))
                C2 = res.tile([1, D], F32R)
                nc.gpsimd.dma_start(C2[:], C2_d[:])
                onesr = res.tile([1, 128], F32R)
                nc.gpsimd.dma_start(onesr[:], onesr_d[:])
                g0b = res.tile([128, D], F32)
                nc.gpsimd.dma_start(g0b[:], g0b_d[:])
                g1b = res.tile([128, D], F32)
                nc.gpsimd.dma_start(g1b[:], g1b_d[:])
                be1b = res.tile([128, D], F32)
                nc.gpsimd.dma_start(be1b[:], be1b_d[:])

            abias = sm[:, 0:8]
            onesc = sm[:, 8:10]
            c1 = sm[:, 10:14]
            epsT = sm[:, 14:15]

            ident = res.tile([128, 128], F32)
            masks.make_identity(nc, ident[:])

            # HAM warm-up: keep PE busy while the first DMAs land.
            for w in range(16):
                psw = psA.tile([128, 128], F32, tag="a", name=f"psw{w}")
                nc.tensor.matmul(psw[:], ident[:], ident[:], start=True, stop=True)

            # big SBUF intermediates; "mid"/"zx" tags reuse slots
            IN2 = res.tile([128, ND, S], F32R, tag="mid")   # M^T x^T
            PT = res.tile([128, NQ, S], F32R)               # exp(scores^T)
            zT = res.tile([128, ND, S], F32R)
            ff1T = res.tile([128, ND, S], F32R)

            # ---- phase 1: IN2[a, q] = sum_b M[b, a] xT[b, q] ----
            for at in range(ND):
                for qc in range(2):
                    ps = psA.tile([128, 512], F32, tag="a")
                    for bt in range(ND):
                        nc.tensor.matmul(
                            ps[:],
                            Mw[:, bt, at * 128:(at + 1) * 128],
                            xT[:, bt, qc * 512:(qc + 1) * 512],
                            start=(bt == 0), stop=(bt == ND - 1),
                        )
                    nc.vector.tensor_copy(IN2[:, at, qc * 512:(qc + 1) * 512], ps[:])

            # ---- phase 2: scoresT[k, q] = sum_a x[k, a] IN2[a, q]; PT = exp ----
            for kt in range(NQ):
                ps = psS.tile([128, 1024], F32, tag="s")
                for qc in range(2):
                    for at in range(ND):
                        nc.tensor.matmul(
                            ps[:, qc * 512:(qc + 1) * 512],
                            xT[:, at, kt * 128:(kt + 1) * 128],
                            IN2[:, at, qc * 512:(qc + 1) * 512],
                            start=(at == 0), stop=(at == ND - 1),
                        )
                bias = 0.0 if fast else abias[:, kt:kt + 1]
                nc.scalar.activation(PT[:, kt, :], ps[:], AF.Exp, bias=bias)

            # denomT[q] = sum_k PT[k, q]: DVE tree-sums the 8 k-tiles, then one
            # cheap f32 ones-matmul per q-tile flips [k-part, q] to [q-part, 1].
            dps = psD.tile([128, 2 * NQ], F32, tag="d")
            for qc in range(2):
                qs = slice(qc * 512, (qc + 1) * 512)
                dacc = work.tile([128, 512], F32, tag="sq", name=f"dacc{qc}")
                nc.vector.tensor_add(dacc[:], PT[:, 0, qs].bitcast(F32), PT[:, 1, qs].bitcast(F32))
                for kt in range(2, NQ):
                    nc.vector.tensor_add(dacc[:], dacc[:], PT[:, kt, qs].bitcast(F32))
                for ql in range(4):
                    qt = qc * 4 + ql
                    nc.tensor.matmul(
                        dps[:, 2 * qt:2 * qt + 2],
                        dacc[:, ql * 128:(ql + 1) * 128],
                        onesc[:],
                        start=True, stop=True,
                    )
            recip = res.tile([128, 2 * NQ], F32)
            nc.vector.reciprocal(recip[:], dps[:])

            # ---- phase 3: ST[d, q] = sum_k x[k, d] PT[k, q] ----
            ST = res.tile([128, ND, S], F32R, tag="mid")
            for dt in range(ND):
                for qc in range(2):
                    ps = psA.tile([128, 512], F32, tag="a")
                    for kt in range(NQ):
                        nc.tensor.matmul(
                            ps[:],
                            xn[:, kt, dt * 128:(dt + 1) * 128],
                            PT[:, kt, qc * 512:(qc + 1) * 512],
                            start=(kt == 0), stop=(kt == NQ - 1),
                        )
                    nc.scalar.copy(ST[:, dt, qc * 512:(qc + 1) * 512], ps[:])

            # ---- phase 4: mha + residual + LN1 -> z  (stage-major pipeline) ----
            z = res.tile([128, NQ, D], F32, tag="zx")
            x1s, lns = [], []
            for qt in range(NQ):
                ps = psA.tile([128, 512], F32, tag="a")
                for dt in range(ND):
                    nc.tensor.matmul(
                        ps[:],
                        ST[:, dt, qt * 128:(qt + 1) * 128],
                        NTw[:, dt, :],
                        start=(dt == 0), stop=(dt == ND - 1),
                    )
                x1 = work.tile([128, D], F32, tag="x1", bufs=NQ, name=f"x1_{qt}")
                nc.scalar.mul(x1[:], ps[:], recip[:, 2 * qt:2 * qt + 1])
                resid = xn[:, qt, :].bitcast(F32) if fast else xres[:, qt, :]
                nc.vector.tensor_add(x1[:], x1[:], resid)
                x1s.append(x1)
                lns.append(_ln_stats(nc, small, work, x1, epsT))
            for qt in range(NQ):
                _ln_apply(nc, small, x1s[qt], lns[qt], z[:, qt, :])

            # ---- phase 5: zT = z^T (PE transposes) ----
            for qt in range(NQ):
                for dt in range(ND):
                    pst = psA.tile([128, 128], F32, tag="a", name="pst")
                    nc.tensor.transpose(
                        pst[:], z[:, qt, dt * 128:(dt + 1) * 128], ident[:]
                    )
                    nc.vector.tensor_copy(zT[:, dt, qt * 128:(qt + 1) * 128], pst[:])

            # ---- phase 6: ff1T[e, q] = relu(sum_d W1gT[d, e] zT[d, q] + c1[e]) ----
            for et in range(ND):
                for qc in range(2):
                    ps = psA.tile([128, 512], F32, tag="a")
                    for dt in range(ND):
                        nc.tensor.matmul(
                            ps[:],
                            W1gT[:, dt, et * 128:(et + 1) * 128],
                            zT[:, dt, qc * 512:(qc + 1) * 512],
                            start=(dt == 0), stop=(dt == ND - 1),
                        )
                    nc.scalar.activation(
                        ff1T[:, et, qc * 512:(qc + 1) * 512], ps[:],
                        AF.Relu, bias=c1[:, et:et + 1],
                    )

            # ---- phase 7: ff2 + residual + LN2 -> out (stage-major pipeline) ----
            rs, lns2 = [], []
            for qt in range(NQ):
                ps = psA.tile([128, 512], F32, tag="a")
                for et in range(ND):
                    nc.tensor.matmul(
                        ps[:],
                        ff1T[:, et, qt * 128:(qt + 1) * 128],
                        W2T[:, et, :],
                        start=(et == 0), stop=(fast and et == ND - 1),
                    )
                if not fast:
                    nc.tensor.matmul(ps[:], onesr[:], C2[:], start=False, stop=True)
                r = work.tile([128, D], F32, tag="r", bufs=NQ, name=f"r_{qt}")
                if fast:
                    nc.vector.tensor_add(r[:], ps[:], z[:, qt, :])
                else:
                    hres = work.tile([128, D], F32, tag="hres")
                    nc.vector.tensor_mul(hres[:], z[:, qt, :], g0b[:])
                    nc.vector.tensor_add(r[:], ps[:], hres[:])
                rs.append(r)
                lns2.append(_ln_stats(nc, small, work, r, epsT))
            for qt in range(NQ):
                od = out_d.rearrange("(t p) n -> p t n", p=128)[:, qt, :]
                z2 = work.tile([128, D], F32, tag="z2")
                _ln_apply(nc, small, rs[qt], lns2[qt], z2[:])
                if fast:
                    nc.sync.dma_start(od, z2[:])
                else:
                    z2g = work.tile([128, D], F32, tag="sq")
                    nc.vector.tensor_mul(z2g[:], z2[:], g1b[:])
                    ot = work.tile([128, D], F32, tag="r2")
                    nc.vector.tensor_add(ot[:], z2g[:], be1b[:])
                    nc.sync.dma_start(od, ot[:])

    nc.compile()
    _BUILT[fast] = (nc,)
    return _BUILT[fast]


def _ln_stats(nc, small, work, x1, epsT):
    """Stats for LN over the free axis: returns (rstd, nmr) [128,1] tiles.
    Uses var = E[x^2] - mean^2 so the Square runs parallel to the row-sum."""
    s1 = small.tile([128, 1], F32, tag="s1")
    nc.vector.reduce_sum(s1[:], x1[:], axis=AX.X)
    sq = work.tile([128, D], F32, tag="sq")
    sqs = small.tile([128, 1], F32, tag="sqs")
    nc.scalar.activation(sq[:], x1[:], AF.Square, bias=0.0, accum_out=sqs[:])
    nm = small.tile([128, 1], F32, tag="nm")
    nc.scalar.mul(nm[:], s1[:], -1.0 / D)
    m2e = small.tile([128, 1], F32, tag="m2e")
    nc.vector.tensor_mul(m2e[:], nm[:], nm[:])
    nc.vector.tensor_scalar(m2e[:], m2e[:], -1.0, None, op0=mybir.AluOpType.mult)
    nc.vector.tensor_add(m2e[:], m2e[:], epsT)
    stdv = small.tile([128, 1], F32, tag="stdv")
    nc.scalar.activation(stdv[:], sqs[:], AF.Sqrt, scale=1.0 / D, bias=m2e[:])
    rstd = small.tile([128, 1], F32, tag="rstd")
    nc.vector.reciprocal(rstd[:], stdv[:])
    nmr = small.tile([128, 1], F32, tag="nmr")
    nc.vector.tensor_mul(nmr[:], nm[:], rstd[:])
    return rstd, nmr


def _ln_apply(nc, small, x1, stats, out_ap):
    rstd, nmr = stats
    nc.scalar.activation(out_ap, x1[:], AF.Identity, scale=rstd[:], bias=nmr[:])


def _prepare_in_maps(inputs):
    f64 = np.float64
    g = {k: np.asarray(v) for k, v in inputs.items()}
    x = g["x"].astype(f64)
    Wq, Wk, Wv = g["Wq"].astype(f64), g["Wk"].astype(f64), g["Wv"].astype(f64)
    Wo, W1, W2 = g["Wo"].astype(f64), g["W1"].astype(f64), g["W2"].astype(f64)
    bq, bk, bv, bo = g["bq"].astype(f64), g["bk"].astype(f64), g["bv"].astype(f64), g["bo"].astype(f64)
    b1, b2 = g["b1"].astype(f64), g["b2"].astype(f64)
    g0, be0, g1, be1 = g["g0"].astype(f64), g["be0"].astype(f64), g["g1"].astype(f64), g["be1"].astype(f64)

    fast = (
        not np.any(bq) and not np.any(bk) and not np.any(bv) and not np.any(bo)
        and not np.any(b1) and not np.any(b2) and not np.any(be0) and not np.any(be1)
        and bool(np.all(g0 == 1.0)) and bool(np.all(g1 == 1.0))
    )

    s = math.sqrt(D)
    f32 = lambda a: np.ascontiguousarray(a, dtype=np.float32)

    Mw = f32(Wq.T @ Wk / s)
    NT = f32((Wo @ Wv).T)
    W1gT = f32((W1 * g0[None, :]).T)
    W2T = f32(W2.T)
    c1 = f32(b1 + W1 @ be0)
    wbo = Wo @ bv + bo
    vk = Wk.T @ bq / s

    shared = dict(Mw=Mw, NT=NT, W1gT=W1gT, W2T=W2T)
    if not fast:
        shared["C2"] = f32(b2 + be0).reshape(1, D)
        shared["onesr"] = np.ones((1, 128), np.float32)
        shared["g0b"] = f32(np.broadcast_to(g0, (128, D)))
        shared["g1b"] = f32(np.broadcast_to(g1, (128, D)))
        shared["be1b"] = f32(np.broadcast_to(be1, (128, D)))

    in_maps = []
    for b in range(B):
        xb = x[b]
        m = dict(shared)
        m["xT"] = f32(xb.T)
        m["xn"] = f32(xb)
        smalls = np.zeros((128, 16), np.float32)
        smalls[:, 0:8] = f32(xb @ vk).reshape(8, 128).T
        smalls[:, 8:10] = 1.0
        smalls[:, 10:14] = c1.reshape(4, 128).T
        smalls[:, 14:15] = 1e-5
        m["smalls"] = smalls
        if not fast:
            m["xres"] = f32(xb + wbo[None, :])
        in_maps.append(m)
    return fast, in_maps


def _run(inputs, trace=False):
    fast, in_maps = _prepare_in_maps(inputs)
    (nc,) = _build(fast)
    res = run_bass_kernel_spmd(nc, in_maps, core_ids=list(range(B)), trace=trace)
    out = np.stack([res.results[c]["out"] for c in range(B)]).astype(np.float32)
    return out, res


def kernel(**inputs):
    out, _ = _run(inputs, trace=False)
    return out
